# revision 48
# baseline (speedup 1.0000x reference)
"""DharmaAttention TRN2 kernel (v2 — single-pass, SBUF-resident, bf16).

Full-input contract: kernel(**inputs) takes the unsharded inputs and returns
the full [2, 2048, 2048] float32 output.

Sharding (8 cores): 2-way data-parallel over batch x 4-way tensor-parallel
over head groups (4 heads of head_dim 128 per core). Wq/Wk/Wv split
column-wise per head group, Wo row-wise; host sums the 4 partials per batch.

v2 design vs the phase-serial baseline:
  - all matmul operands bf16 (PE streams ~1.2 cyc/row either way, but DMA
    halves and DVE gets the 2x packed mode)
  - ONE pass over x: Q/K/V projected per 512-seq chunk; q/k/v/outh are
    SBUF-resident (no DRAM roundtrip between phases)
  - software-pipelined program order
        proj(0) proj(1) [attn(0) op(0)] proj(2) [attn(1) op(1)] proj(3)
        [attn(2) op(2)] [attn(3) op(3)]
    so the PE never waits on rope/exp latency
  - softmax denominator: DVE running-sum of exp tiles into fp16 (replaces
    the baseline's 160 all-ones matmuls with 16) + reciprocal_approx_fast
  - exp computed as exp(s/sqrt(128) - 4*ln2): the 1/16 scale cancels in the
    softmax ratio and keeps the fp16 denominator far from overflow
  - PSUM->SBUF copies ride the otherwise-idle Pool (gpsimd) engine

Per-core DRAM inputs (host-side prep):
  xs    [128, 4, 16, 512] bf16   x chunk-swizzled: [p, sc, t, j] =
                                 hidden[b][sc*512+j, t*128+p]
  wq,wk [128, 16, 512]    bf16   [p, t, o] = W[g*512+o, t*128+p]
  wv    [128, 16, 512]    bf16   same layout as wq/wk
  wo    [128, 4, 2048]    bf16   [p, c, o] = Wo[o, g*512 + c*128 + p]
  cosT  [128, 2048]       bf16   rope cos, [d, s]
  sinN  [128, 2048]       bf16   rows 0:64 = -sin, 64:128 = +sin
  maskd [128, 4, 512]     bf16   causal masks for the 4 diagonal offsets
Output:
  yT    [2048, 2048]      bf16   partial (Wo row-shard) output, [o, s]
"""

import math
import sys

sys.path.insert(0, "/opt/trn_rl_repo")

import numpy as np

B = 2
S = 2048
H = 2048
NH = 16
HD = 128
THETA = 10000.0
G = 4  # heads per core
GC = G * HD  # 512 channels per core
NHT = H // 128  # 16 contraction tiles
SC = 512  # seq chunk
NSC = S // SC  # 4
INV_SQRT_HD = 1.0 / math.sqrt(HD)
EXP_BIAS = -4.0 * math.log(2.0)  # exp scaled by 1/16; cancels in softmax

_prog_cache = {}

# test-harness hooks (the grading path leaves these at defaults)
TRACE = False
LAST_RESULTS = None


def _split_multi_waits(nc):
    """The walrus build here accepts at most ONE sync wait per instruction
    ('Too many sync wait commands'). Hoist extra on_wait entries into no-op
    instructions inserted just before, on the same engine."""
    import concourse.mybir as mybir

    for f in nc.m.functions:
        for b in f.blocks:
            out = []
            changed = False
            for inst in b.instructions:
                si = getattr(inst, "sync_info", None)
                waits = list(si.on_wait) if si is not None and si.on_wait else []
                if len(waits) > 1:
                    for k, w in enumerate(waits[:-1]):
                        nop = mybir.InstNoOp(
                            name=f"{inst.name}-w{k}",
                            sync_info=mybir.SyncInfo(on_wait=[w], on_update=[]),
                        )
                        nop.engine = inst.engine
                        out.append(nop)
                    inst.sync_info = mybir.SyncInfo(
                        on_wait=[waits[-1]], on_update=list(si.on_update or [])
                    )
                    changed = True
                out.append(inst)
            if changed:
                b.instructions = out


def _build_nc():
    import concourse.bass as bass
    import concourse.mybir as mybir
    import concourse.tile as tile

    F32 = mybir.dt.float32
    F16 = mybir.dt.float16
    BF16 = mybir.dt.bfloat16
    MULT = mybir.AluOpType.mult
    ADD = mybir.AluOpType.add
    EXP = mybir.ActivationFunctionType.Exp
    LN = mybir.ActivationFunctionType.Ln

    nc = bass.Bass("TRN2", target_bir_lowering=False, debug=False)

    xs_d = nc.dram_tensor("xs", [128, NSC, NHT, SC], BF16, kind="ExternalInput").ap()
    wq_d = nc.dram_tensor("wq", [128, NHT, GC], BF16, kind="ExternalInput").ap()
    wk_d = nc.dram_tensor("wk", [128, NHT, GC], BF16, kind="ExternalInput").ap()
    wv_d = nc.dram_tensor("wv", [128, NHT, GC], BF16, kind="ExternalInput").ap()
    wo_d = nc.dram_tensor("wo", [128, G, H], BF16, kind="ExternalInput").ap()
    cosT_d = nc.dram_tensor("cosT", [HD, S], BF16, kind="ExternalInput").ap()
    sinN_d = nc.dram_tensor("sinN", [HD, S], BF16, kind="ExternalInput").ap()
    maskd_d = nc.dram_tensor("maskd", [128, 128], BF16, kind="ExternalInput").ap()
    yT = nc.dram_tensor("yT", [H, S], BF16, kind="ExternalOutput").ap()

    with tile.TileContext(nc) as tc:
        with (
            tc.tile_pool(name="consts", bufs=1) as consts,
            tc.tile_pool(name="xpool", bufs=2) as xpool,
            tc.tile_pool(name="qkv", bufs=1) as qkv,
            tc.tile_pool(name="qch", bufs=2) as qpool,
            tc.tile_pool(name="ohch", bufs=2) as ohpool,
            tc.tile_pool(name="rp", bufs=2) as rpool,
            tc.tile_pool(name="pr", bufs=3) as prpool,
            tc.tile_pool(name="psm", bufs=2) as psmpool,
            tc.tile_pool(name="bc", bufs=1) as bcpool,
            tc.tile_pool(name="ys", bufs=2) as ypool,
            tc.tile_pool(name="ps", bufs=1, space="PSUM") as ps,
        ):
            cosT = consts.tile([HD, S], BF16)
            sinN = consts.tile([HD, S], BF16)
            maskd = consts.tile([128, 128], BF16)
            ones16 = consts.tile([128, 128], F16)
            wq_sb = consts.tile([128, NHT, GC], BF16)
            wk_sb = consts.tile([128, NHT, GC], BF16)
            wv_sb = consts.tile([128, NHT, GC], BF16)
            wo_sb = consts.tile([128, G, H], BF16)
            x_sb0 = xpool.tile([128, NHT, SC], BF16)
            # DMA order = need order: x(0)/wq pieces first so the first
            # matmul group starts within a few us; wo/mask deferred.
            for p in range(4):
                hsl = slice(4 * p, 4 * p + 4)
                nc.sync.dma_start(out=x_sb0[:, hsl, :], in_=xs_d[:, 0, hsl, :])
                nc.sync.dma_start(out=wq_sb[:, hsl, :], in_=wq_d[:, hsl, :])
                if p == 0:
                    nc.sync.dma_start(out=cosT, in_=cosT_d)
                elif p == 1:
                    nc.sync.dma_start(out=sinN, in_=sinN_d)
                else:
                    nc.sync.dma_start(
                        out=wk_sb[:, 4 * p - 8 : 4 * p - 4, :],
                        in_=wk_d[:, 4 * p - 8 : 4 * p - 4, :],
                    )
            for p in range(2, 4):
                hsl = slice(4 * p, 4 * p + 4)
                nc.sync.dma_start(out=wk_sb[:, hsl, :], in_=wk_d[:, hsl, :])
            nc.sync.dma_start(out=wv_sb, in_=wv_d)
            nc.sync.dma_start(out=maskd, in_=maskd_d)
            nc.sync.dma_start(out=wo_sb, in_=wo_d)
            nc.vector.memset(ones16, 1.0)
            ebias = consts.tile([128, 1], F32)
            nc.vector.memset(ebias, EXP_BIAS)

            k_sb = qkv.tile([128, G, S], BF16)
            v_sb = qkv.tile([128, NHT, GC], BF16)  # [kpos, kb, och]

            def proj(sc):
                ssl = slice(sc * SC, (sc + 1) * SC)
                if sc == 0:
                    x_sb = x_sb0
                else:
                    x_sb = xpool.tile([128, NHT, SC], BF16)
                    nc.sync.dma_start(out=x_sb, in_=xs_d[:, sc])
                # Q/K projection + rope; all Q heads first so the wk DMA has
                # time to land during chunk 0
                q_ch = qpool.tile([128, G, SC], BF16)
                for w_sb, dst in ((wq_sb, None), (wk_sb, k_sb)):
                    for h in range(G):
                        osl = slice(h * 128, (h + 1) * 128)
                        pp = ps.tile([128, SC], F32, tag="pa", bufs=2)
                        for ht in range(NHT):
                            nc.tensor.matmul(
                                pp,
                                w_sb[:, ht, osl],
                                x_sb[:, ht, :],
                                start=(ht == 0),
                                stop=(ht == NHT - 1),
                            )
                        # rope: rotate-half as partition-shifted PSUM copies
                        # on the Act engine, then 2x-mode DVE muls:
                        # dst = pp*cos + pxr*sinN (sinN rows 0:64 = -sin)
                        pxr = rpool.tile([128, SC], BF16, tag="pxr")
                        nc.scalar.copy(pxr[0:64, :], pp[64:128, :])
                        nc.scalar.copy(pxr[64:128, :], pp[0:64, :])
                        tmp = rpool.tile([128, SC], BF16, tag="tmp")
                        nc.vector.tensor_tensor(
                            out=tmp, in0=pxr, in1=sinN[:, ssl], op=MULT
                        )
                        cp = rpool.tile([128, SC], BF16, tag="cp")
                        nc.vector.tensor_tensor(
                            out=cp, in0=pp, in1=cosT[:, ssl], op=MULT
                        )
                        dap = q_ch[:, h, :] if dst is None else dst[:, h, ssl]
                        nc.vector.tensor_tensor(out=dap, in0=cp, in1=tmp, op=ADD)
                # V projection: x as lhsT so v lands [kpos, och]
                for st2 in range(SC // 128):
                    pv = ps.tile([128, GC], F32, tag="pa", bufs=2)
                    for ht in range(NHT):
                        nc.tensor.matmul(
                            pv,
                            x_sb[:, ht, st2 * 128 : (st2 + 1) * 128],
                            wv_sb[:, ht, :],
                            start=(ht == 0),
                            stop=(ht == NHT - 1),
                        )
                    nc.scalar.copy(v_sb[:, sc * 4 + st2, :], pv)
                return q_ch

            def attn(qc, q_ch):
                oh = ohpool.tile([128, G, SC], BF16)
                nk = 4 * qc + 4
                # diag blocks first: m=0 (full width) opens the PSUM group,
                # the partial-width diag blocks sit in the middle, and a
                # full-width non-diag block closes it (keeps the sim's
                # accumulation-group tracking happy). qc=0 has only diag
                # blocks, so there the PV stays full-width with a zeroed
                # prefix.
                kseq = list(range(4 * qc, nk)) + list(range(0, 4 * qc))
                for h in range(G):
                    osl = slice(h * 128, (h + 1) * 128)
                    po = ps.tile([128, SC], F32, tag="po", bufs=3)
                    prsum = psmpool.tile([128, SC], F16)
                    for idx, ki in enumerate(kseq):
                        psc = ps.tile([128, SC], F32, tag="psc", bufs=3)
                        nc.tensor.matmul(
                            psc,
                            k_sb[:, h, ki * 128 : (ki + 1) * 128],
                            q_ch[:, h, :],
                            start=True,
                            stop=True,
                        )
                        pr = prpool.tile([128, SC], BF16)
                        m = ki - 4 * qc
                        # diag block at offset m > 0: cols < 128m are fully
                        # causal-masked -> exp/sum only the suffix
                        a = 128 * m if m > 0 else 0
                        if a and qc == 0:
                            nc.vector.memset(pr[:, 0:a], 0.0)
                        nc.scalar.activation(
                            pr[:, a:], psc[:, a:], EXP,
                            scale=INV_SQRT_HD, bias=ebias,
                        )
                        if m >= 0:
                            # triangle mask on the 128-wide diagonal sub-block
                            dsl = slice(128 * m, 128 * m + 128)
                            nc.vector.tensor_tensor(
                                out=pr[:, dsl], in0=pr[:, dsl],
                                in1=maskd, op=MULT,
                            )
                        apv = 0 if qc == 0 else a
                        nc.tensor.matmul(
                            po[:, apv:],
                            v_sb[:, ki, osl],
                            pr[:, apv:],
                            start=(idx == 0),
                            stop=(idx == nk - 1),
                            skip_group_check=(apv > 0),
                        )
                        if idx == 0:
                            nc.vector.tensor_copy(prsum, pr)
                        else:
                            nc.vector.tensor_tensor(
                                out=prsum[:, a:], in0=prsum[:, a:],
                                in1=pr[:, a:], op=ADD,
                            )
                    pbs = ps.tile([128, SC], F32, tag="po", bufs=3)
                    nc.tensor.matmul(pbs, ones16, prsum, start=True, stop=True)
                    # 1/den as exp(-ln(den)) on the Act engine: ~5x cheaper
                    # than DVE reciprocal and off the DVE queue (Exp, Ln and
                    # Copy share one act table -> no reloads)
                    lnb = bcpool.tile([128, SC], F32, tag="lnb")
                    nc.scalar.activation(lnb, pbs, LN)
                    bc = bcpool.tile([128, SC], F32, tag="bc")
                    nc.scalar.activation(bc, lnb, EXP, scale=-1.0)
                    nc.vector.tensor_tensor(
                        out=oh[:, h, :], in0=po, in1=bc, op=MULT
                    )
                return oh

            def outproj(sc, oh):
                qsl = slice(sc * SC, (sc + 1) * SC)
                for ot in range(NHT):
                    py = ps.tile([128, SC], F32, tag="pa", bufs=2)
                    for h in range(G):
                        nc.tensor.matmul(
                            py,
                            wo_sb[:, h, ot * 128 : (ot + 1) * 128],
                            oh[:, h, :],
                            start=(h == 0),
                            stop=(h == G - 1),
                        )
                    ysf = ypool.tile([128, SC], BF16)
                    if ot % 2 == 0:
                        nc.scalar.copy(ysf, py)
                    else:
                        nc.vector.tensor_copy(ysf, py)
                    nc.sync.dma_start(
                        out=yT[ot * 128 : (ot + 1) * 128, qsl], in_=ysf
                    )

            q0 = proj(0)
            q1 = proj(1)
            oh0 = attn(0, q0)
            outproj(0, oh0)
            q2 = proj(2)
            oh1 = attn(1, q1)
            outproj(1, oh1)
            q3 = proj(3)
            oh2 = attn(2, q2)
            outproj(2, oh2)
            oh3 = attn(3, q3)
            outproj(3, oh3)

    _split_multi_waits(nc)
    return nc


def _host_tables():
    import ml_dtypes

    BF = ml_dtypes.bfloat16
    inv_freq = 1.0 / (THETA ** (np.arange(0, HD, 2, dtype=np.float32) / HD))
    t = np.arange(S, dtype=np.float32)
    freqs = np.einsum("i,j->ij", t, inv_freq)  # [S, 64]
    cos_h = np.cos(freqs).astype(np.float32)
    sin_h = np.sin(freqs).astype(np.float32)
    cosT = np.empty((HD, S), np.float32)
    cosT[0:64] = cos_h.T
    cosT[64:128] = cos_h.T
    sinN = np.empty((HD, S), np.float32)
    sinN[0:64] = -sin_h.T
    sinN[64:128] = sin_h.T
    p = np.arange(128)[:, None]
    s = np.arange(128)[None, :]
    maskd = (s >= p).astype(np.float32)  # [128, 128] causal triangle
    return cosT.astype(BF), sinN.astype(BF), maskd.astype(BF)


def _prep_inputs(hidden_states, Wq, Wk, Wv, Wo):
    """Per-core input maps: convert to bf16 and pre-swizzle for contiguous
    per-partition DMA lines."""
    import ml_dtypes

    BF = ml_dtypes.bfloat16
    cosT, sinN, maskd = _host_tables()
    xs_b = []
    for b in range(B):
        xT = np.ascontiguousarray(hidden_states[b].T).astype(BF)  # [H, S]
        # [p, sc, t, j] = xT[t*128+p, sc*512+j]
        xs = xT.reshape(NHT, 128, NSC, SC).transpose(1, 2, 0, 3)
        xs_b.append(np.ascontiguousarray(xs))
    in_maps = []
    for c in range(8):
        b, g = divmod(c, 4)
        rows = slice(g * GC, (g + 1) * GC)

        def wsw(W):
            # [p, t, o] = W[rows][o, t*128+p].T ; W[rows] is [512, 2048]
            wT = np.ascontiguousarray(W[rows, :].T).astype(BF)  # [2048in, 512]
            return np.ascontiguousarray(
                wT.reshape(NHT, 128, GC).transpose(1, 0, 2)
            )

        woT = np.ascontiguousarray(Wo[:, rows].T).astype(BF)  # [512c, 2048o]
        wo = np.ascontiguousarray(woT.reshape(G, 128, H).transpose(1, 0, 2))
        in_maps.append(
            {
                "xs": xs_b[b],
                "wq": wsw(Wq),
                "wk": wsw(Wk),
                "wv": wsw(Wv),
                "wo": wo,
                "cosT": cosT,
                "sinN": sinN,
                "maskd": maskd,
            }
        )
    return in_maps


def kernel(hidden_states, Wq, Wk, Wv, Wo):
    from concourse import bass_utils

    hidden_states = np.asarray(hidden_states, dtype=np.float32)
    Wq = np.asarray(Wq, dtype=np.float32)
    Wk = np.asarray(Wk, dtype=np.float32)
    Wv = np.asarray(Wv, dtype=np.float32)
    Wo = np.asarray(Wo, dtype=np.float32)

    if "nc" not in _prog_cache:
        _prog_cache["nc"] = _build_nc()
    nc = _prog_cache["nc"]

    in_maps = _prep_inputs(hidden_states, Wq, Wk, Wv, Wo)

    res = bass_utils.run_bass_kernel_spmd(
        nc, in_maps, core_ids=list(range(8)), trace=TRACE
    )
    global LAST_RESULTS
    LAST_RESULTS = res

    out = np.zeros((B, S, H), np.float32)
    for c in range(8):
        b = c // 4
        out[b] += res.results[c]["yT"].T.astype(np.float32)
    return out
